# revision 19
# baseline (speedup 1.0000x reference)
"""Multi-head causal attention (b=1, s=4096, d=1024, 16 heads) on 8 NeuronCores.

Sharding: tensor-parallel over heads - 2 heads per core. Each core computes
Q/K/V projections for its heads, causal attention, and its row-slice of the
output projection (partial sum). Host sums the 8 partial outputs (bf16
partials, fp32 accumulate).

v2 layout notes (vs the v1 baseline):
 - PV is computed FLIPPED: out O[q, d] chunks [128, 65] with the probs tile
   e [keys, queries] as the stationary operand and V_aug [keys, 65] as the
   moving operand. The cost model charges matmuls by moving-free-dim rows,
   so 4x65-row matmuls beat one 512-row matmul ~2x. The ones column of
   V_aug lands the softmax denominator in the O tile for free.
 - O accumulates in PSUM across all k-tiles of a query chunk; the PSUM
   banks are pre-zeroed by a DVE memset and every PV matmul uses
   start=False, so no start=True ever wipes a bank shared by the 8
   accumulator slices (start=True marks the whole 2KB bank pending-zero).
 - O is normalized by DVE tensor_scalar muls (per-partition reciprocal of
   the denominator column), then transposed back to O^T via PE transpose
   (identity moving operand) for the Wo matmul.
 - Scores matmuls and exp are emitted under tc.high_priority() so the
   scheduler never lets projection/Wo filler work delay the ACT exp
   cadence (exp is the binding resource at ~140us busy); Wo output
   copies+DMAs are deprioritized into engine gaps, with the last chunk's
   copies drained on the scalar engine while it is otherwise idle.
 - Causal masking multiplies only the [128,128] diagonal blocks of e.
 - exp has no max-subtraction: scores ~ N(0,1) by construction, fp32 PSUM
   holds exp(s/8) easily.
 - y is written bf16 (halves DMA-out); host accumulates partials in fp32.
"""

import numpy as np
import ml_dtypes

import concourse.bass as bass
import concourse.mybir as mybir
import concourse.tile as tile
from concourse import bacc
from concourse.bass_utils import run_bass_kernel_spmd

BF16 = ml_dtypes.bfloat16
S = 4096          # sequence length
D = 1024          # model dim
NCORES = 8
HL = 2            # heads per core
HD = 64           # head dim
DK = D // 128     # 8 contraction tiles for projections
NQC = S // 512    # 8 query chunks of 512
NKT = S // 128    # 32 key tiles of 128
FP32 = mybir.dt.float32
BF = mybir.dt.bfloat16
EXP = mybir.ActivationFunctionType.Exp


def _build_program(repeat=1):
    nc = bacc.Bacc("TRN2", target_bir_lowering=False, debug=False, num_devices=NCORES)

    xT = nc.dram_tensor("xT", [D, S], BF, kind="ExternalInput").ap()
    wq = nc.dram_tensor("wq", [D, 128], BF, kind="ExternalInput").ap()
    wk = nc.dram_tensor("wk", [D, 128], BF, kind="ExternalInput").ap()
    wv = nc.dram_tensor("wv", [D, 128], BF, kind="ExternalInput").ap()
    wo = nc.dram_tensor("wo", [128, D], BF, kind="ExternalInput").ap()
    # tri[k, c] = 1 where k <= c (keep), 0 above the diagonal
    tri = nc.dram_tensor("tri", [128, 128], BF, kind="ExternalInput").ap()
    ident = nc.dram_tensor("ident", [128, 128], BF, kind="ExternalInput").ap()
    y = nc.dram_tensor("y", [S, D], BF, kind="ExternalOutput").ap()

    with tile.TileContext(nc) as tc:
        with (
            tc.tile_pool(name="persist", bufs=1) as pp,
            tc.tile_pool(name="stp", bufs=2, space="PSUM") as stp,
            tc.tile_pool(name="sdp", bufs=2, space="PSUM") as sdp,
            tc.tile_pool(name="otp", bufs=1, space="PSUM") as otp,
            tc.tile_pool(name="epool", bufs=14) as ep,
            tc.tile_pool(name="small", bufs=4) as sp,
            tc.tile_pool(name="onp", bufs=4) as onp,
            tc.tile_pool(name="ystage", bufs=10) as ysp,
        ):
            # ---- persistent SBUF tiles ----
            # chunk c holds all 8 D-row-blocks of xT for seq cols
            # [512c, 512c+512): block i at free cols [512i, 512i+512)
            xt = [pp.tile([128, DK * 512], BF, tag=f"xt{c}", name=f"xt{c}")
                  for c in range(NQC)]
            wq_sb = pp.tile([128, D], BF, tag="wq")
            wk_sb = pp.tile([128, D], BF, tag="wk")
            wv_sb = pp.tile([128, D], BF, tag="wv")
            wo_sb = pp.tile([128, D], BF, tag="wo")
            tri_sb = pp.tile([128, 128], BF, tag="tri")
            id_sb = pp.tile([128, 128], BF, tag="ident")
            qT = [pp.tile([128, 512], BF, tag=f"qT{c}", name=f"qT{c}") for c in range(NQC)]
            kT = [pp.tile([128, 512], BF, tag=f"kT{c}", name=f"kT{c}") for c in range(NQC)]
            # V augmented: per k-tile layout [V_h0 (64) | ones (1) | V_h1 (64)]
            # h0 moving slice = cols 0:65   -> O cols 0..63 = V, col 64 = den
            # h1 moving slice = cols 64:129 -> O col 0 = den, cols 1..64 = V
            # grouped 4 k-tiles per tile: k-tile kt at cols 129*(kt%4)
            vaug = [pp.tile([128, 4 * 129], BF, tag=f"va{g}", name=f"va{g}")
                    for g in range(NKT // 4)]
            # normalized attention output O^T, chunked by query chunk
            ot_sb = [pp.tile([128, 512], BF, tag=f"ot{c}", name=f"ot{c}") for c in range(NQC)]

            # ---- input DMAs: weights first (small, needed immediately) ----
            def load_w(w_sb, w_dram):
                nc.sync.dma_start(
                    out=w_sb[:].rearrange("p (i j) -> p i j", i=DK),
                    in_=w_dram.rearrange("(i p) j -> p i j", p=128),
                )

            xTr = xT.rearrange("(i p) s -> p i s", p=128)

            def load_x_chunk(c, split=False):
                if split:
                    # per-D-tile DMAs so the first projection's matmul i can
                    # start as soon as block i lands
                    for i in range(DK):
                        nc.sync.dma_start(
                            out=xt[c][:, 512 * i:512 * (i + 1)],
                            in_=xT[128 * i:128 * (i + 1), 512 * c:512 * (c + 1)],
                        )
                else:
                    nc.sync.dma_start(
                        out=xt[c][:].rearrange("p (i s) -> p i s", i=DK),
                        in_=xTr[:, :, 512 * c:512 * (c + 1)],
                    )

            load_w(wk_sb, wk)
            load_x_chunk(0, split=True)
            load_w(wq_sb, wq)
            load_x_chunk(1)
            load_w(wv_sb, wv)
            nc.sync.dma_start(out=tri_sb[:], in_=tri[:])
            nc.sync.dma_start(out=id_sb[:], in_=ident[:])
            nc.sync.dma_start(out=wo_sb[:], in_=wo[:])
            load_x_chunk(2)
            load_x_chunk(3)
            load_x_chunk(4)

            # ones columns of vaug
            for g in range(NKT // 4):
                for j in range(4):
                    nc.gpsimd.memset(vaug[g][:, j * 129 + 64:j * 129 + 65], 1.0)

            # ---- per-chunk projections (emitted interleaved with attention) ----
            _qk_accs = {}

            def qk_half(qc, which, half):
                w_sb, dst = (wk_sb, kT[qc]) if which == "k" else (wq_sb, qT[qc])
                if half == 0:
                    acc = sdp.tile([128, 512], FP32, tag="sd", name="mmt")
                    _qk_accs[(qc, which)] = acc
                else:
                    acc = _qk_accs.pop((qc, which))
                for i in range(4 * half, 4 * half + 4):
                    nc.tensor.matmul(
                        acc[:],
                        w_sb[:, 128 * i:128 * (i + 1)],
                        xt[qc][:, 512 * i:512 * (i + 1)],
                        start=(i == 0),
                        stop=(i == DK - 1),
                    )
                if half == 1:
                    nc.vector.tensor_copy(dst[:], acc[:])

            def v_group(kt):
                g = kt // 4
                acc = sdp.tile([128, 128], FP32, tag="sd", name="mmv")
                for i in range(DK):
                    nc.tensor.matmul(
                        acc[:],
                        xt[g][:, 512 * i + 128 * (kt % 4):512 * i + 128 * (kt % 4) + 128],
                        wv_sb[:, 128 * i:128 * (i + 1)],
                        start=(i == 0),
                        stop=(i == DK - 1),
                    )
                j = kt % 4
                # two heads' V slices into the vaug layout (Pool engine)
                nc.vector.tensor_copy(vaug[g][:, j * 129:j * 129 + 64], acc[:, 0:64])
                nc.vector.tensor_copy(vaug[g][:, j * 129 + 65:j * 129 + 129], acc[:, 64:128])

            def proj_units(qc):
                units = [lambda w=w, hf=hf: qk_half(qc, w, hf)
                         for w in ("k", "q") for hf in (0, 1)]
                units += [lambda kt=kt: v_group(kt) for kt in range(4 * qc, 4 * qc + 4)]
                return units

            def vslice(h, kt):
                base = (kt % 4) * 129 + 64 * h
                return vaug[kt // 4][:, base:base + 65]

            def kslice(h, kt):
                return kT[kt // 4][
                    64 * h:64 * h + 64, 128 * (kt % 4):128 * (kt % 4) + 128
                ]

            # ---- causal attention for one query chunk, both heads ----
            def attention(qc, fillers, inline_wo=False):
                # O accumulator: 8 slices of [128, 65] in 2 PSUM banks
                # h0 qslice j at cols 65j (bank 0), h1 at 512 + 65j (bank 1)
                oacc = otp.tile([128, 1024], FP32, tag="ot", name="oacc")
                nc.vector.memset(oacc[:], 0.0)
                st_tiles = {}
                e_tiles = {}
                # count of remaining PV matmuls per O slice, to set stop=
                remaining = {(h, j): (4 * qc + j + 1) for h in range(HL)
                             for j in range(4)}

                def o_out(h, j):
                    return oacc[:, 512 * h + 65 * j:512 * h + 65 * j + 65]

                def pv_one(h, kt, j, ecols):
                    remaining[(h, j)] -= 1
                    nc.tensor.matmul(
                        o_out(h, j),
                        ecols,
                        vslice(h, kt),
                        start=False,
                        stop=(remaining[(h, j)] == 0),
                        skip_group_check=True,
                    )

                # --- full (non-diagonal) k-tile pairs ---
                def s_pair(h, p):
                    st = stp.tile([128, 1024], FP32, tag="st", name="stt")
                    with tc.high_priority():
                        for u in range(2):
                            nc.tensor.matmul(
                                st[:, 512 * u:512 * (u + 1)],
                                kslice(h, 2 * p + u),
                                qT[qc][64 * h:64 * h + 64, :],
                                start=True,
                                stop=True,
                            )
                    st_tiles[(h, p)] = st

                def exp_pair(h, p):
                    e = ep.tile([128, 1024], BF, tag="e", name="etile")
                    with tc.high_priority():
                        nc.scalar.activation(e[:], st_tiles.pop((h, p))[:], EXP,
                                             scale=0.125)
                    e_tiles[(h, p)] = e

                def pv_pair(h, p):
                    e = e_tiles.pop((h, p))
                    for u in range(2):
                        kt = 2 * p + u
                        for j in range(4):
                            pv_one(h, kt, j, e[:, 512 * u + 128 * j:512 * u + 128 * j + 128])

                # --- diagonal k-tile pairs, q-trimmed ---
                # dpair dp covers diag k-tiles j0 = 2dp, 2dp+1 with widths
                # w = 512 - 128*j0 (queries from the diagonal onward)
                def dwidths(dp):
                    j0 = 2 * dp
                    return j0, 512 - 128 * j0, 512 - 128 * (j0 + 1)

                def s_dpair(h, dp):
                    j0, w0, w1 = dwidths(dp)
                    st = stp.tile([128, w0 + w1], FP32, tag="st", name="stdp")
                    with tc.high_priority():
                        nc.tensor.matmul(
                            st[:, 0:w0],
                            kslice(h, 4 * qc + j0),
                            qT[qc][64 * h:64 * h + 64, 512 - w0:512],
                            start=True,
                            stop=True,
                        )
                        nc.tensor.matmul(
                            st[:, w0:w0 + w1],
                            kslice(h, 4 * qc + j0 + 1),
                            qT[qc][64 * h:64 * h + 64, 512 - w1:512],
                            start=True,
                            stop=True,
                        )
                    st_tiles[(h, "d", dp)] = st

                def exp_dpair(h, dp):
                    j0, w0, w1 = dwidths(dp)
                    e = ep.tile([128, w0 + w1], BF, tag="e", name="etiled")
                    with tc.high_priority():
                        nc.scalar.activation(e[:], st_tiles.pop((h, "d", dp))[:],
                                             EXP, scale=0.125)
                    # causal mask: only the leading [128,128] diagonal block
                    # of each diag k-tile needs it
                    nc.gpsimd.tensor_mul(e[:, 0:128], e[:, 0:128], tri_sb[:])
                    nc.gpsimd.tensor_mul(e[:, w0:w0 + 128],
                                         e[:, w0:w0 + 128], tri_sb[:])
                    e_tiles[(h, "d", dp)] = e

                def pv_dpair(h, dp):
                    j0, w0, w1 = dwidths(dp)
                    e = e_tiles.pop((h, "d", dp))
                    # unmasked qslices first (descending j), masked diagonal
                    # block last so the Pool mask muls have time to land
                    units = []
                    for u in range(2):
                        kt = 4 * qc + j0 + u
                        base = w0 * u
                        for j in range(j0 + u, 4):
                            units.append((j == j0 + u, h, kt, j,
                                          base + 128 * (j - j0 - u)))
                    units.sort(key=lambda t: (t[0], -t[3]))
                    # keep stop= ordering correct: emission order here must
                    # match the remaining-count bookkeeping, which only needs
                    # the per-slice last matmul to be emitted last; within a
                    # dpair each slice appears at most twice (u=0 and u=1)
                    for masked, hh, kt, j, off in units:
                        pv_one(hh, kt, j, e[:, off:off + 128])

                # --- normalization + transpose back to O^T ---
                def norm_head(h):
                    # reciprocals of the 4 denominator columns of head h
                    # (h0 den at col 65j+64, h1 den at col 512+65j)
                    rd = sp.tile([128, 4], FP32, tag=f"rd{h}", name="rdt")
                    off0 = 512 * h + 64 * (1 - h)
                    den = oacc[:, off0:off0 + 260].rearrange(
                        "p (i j) -> p i j", j=65)[:, :, 0:1]
                    nc.vector.reciprocal(
                        rd[:].rearrange("p (i o) -> p i o", o=1), den)
                    return rd

                def norm_mul(h, j, rd, onorm):
                    # O slice [128, 64] * recip -> Onorm bf16 cols 64h..64h+64
                    # (h0 V at cols 65j+0..64, h1 V at cols 512+65j+1..65)
                    off = 512 * h + 65 * j + h
                    nc.vector.tensor_scalar_mul(
                        onorm[:, 64 * h:64 * h + 64],
                        oacc[:, off:off + 64],
                        rd[:, j:j + 1],
                    )

                def transpose_j(j, onorm):
                    tp = sdp.tile([128, 128], BF, tag="sd", name="ttile")
                    nc.tensor.transpose(tp[:], onorm[:], id_sb[:])
                    nc.vector.tensor_copy(ot_sb[qc][:, 128 * j:128 * (j + 1)], tp[:])

                # --- emission: diag pairs then full pairs, heads interleaved
                stages = []
                for dp in range(2):
                    stages.append(("d", 0, dp))
                    stages.append(("d", 1, dp))
                for p in range(2 * qc):
                    stages.append(("p", 0, p))
                    stages.append(("p", 1, p))
                emit_s = {"p": s_pair, "d": s_dpair}
                emit_e = {"p": exp_pair, "d": exp_dpair}
                emit_v = {"p": pv_pair, "d": pv_dpair}

                nstages = len(stages)
                for i, (kind, hh, idx) in enumerate(stages):
                    emit_s[kind](hh, idx)
                    if i > 0:
                        pk, ph, pi = stages[i - 1]
                        emit_v[pk](ph, pi)
                    emit_e[kind](hh, idx)
                    # spread remaining fillers evenly over remaining stages
                    rem = nstages - i
                    if fillers and len(fillers) >= rem:
                        for _ in range(-(-len(fillers) // rem)):
                            if fillers:
                                fillers.pop(0)()
                    elif fillers and (i * len(fillers)) // nstages != ((i + 1) * len(fillers)) // nstages:
                        fillers.pop(0)()
                lk, lh, li = stages[-1]
                emit_v[lk](lh, li)

                # normalize + transpose
                onorms = [onp.tile([128, 128], BF, tag="on", name="onorm")
                          for _ in range(4)]
                for h in range(HL):
                    rd = norm_head(h)
                    for j in range(4):
                        norm_mul(h, j, rd, onorms[j])
                for j in range(4):
                    transpose_j(j, onorms[j])
                    if inline_wo:
                        wo_unit(qc, j, 0)
                        wo_unit(qc, j, 1)

            def wo_unit(qc, t, n):
                qt = 4 * qc + t
                acc = sdp.tile([128, 512], FP32, tag="sd", name="yacc")
                nc.tensor.matmul(
                    acc[:],
                    ot_sb[qc][:, 128 * t:128 * (t + 1)],
                    wo_sb[:, 512 * n:512 * (n + 1)],
                    start=True,
                    stop=True,
                )
                ys = ysp.tile([128, 512], BF, tag="ys", name="yst")
                saved = tc.cur_priority
                tc.cur_priority = saved + 1_000_000
                if qc == NQC - 1:
                    # tail: ACT is idle after the last exp, drain there
                    nc.scalar.copy(ys[:], acc[:])
                else:
                    nc.vector.tensor_copy(ys[:], acc[:])
                nc.sync.dma_start(
                    out=y[128 * qt:128 * (qt + 1), 512 * n:512 * (n + 1)],
                    in_=ys[:],
                )
                tc.cur_priority = saved + 1

            def wo_units(qc):
                return [lambda t=t, n=n: wo_unit(qc, t, n)
                        for t in range(4) for n in range(2)]

            for _rep in range(repeat):
              for u in proj_units(0)[:4]:   # k, q halves only
                  u()
              wo_backlog = []
              for qc in range(NQC):
                  if qc + 5 < NQC:
                      load_x_chunk(qc + 5)   # prefetch 5 chunks ahead
                  fillers = []
                  if qc == 0:
                      fillers += proj_units(0)[4:]   # chunk-0 V groups
                  if qc + 1 < NQC:
                      fillers += proj_units(qc + 1)
                  if qc >= 1:
                      wo_backlog += wo_units(qc - 1)
                  if qc >= 5:
                      fillers += wo_backlog
                      wo_backlog = []
                  attention(qc, fillers, inline_wo=(qc == NQC - 1))
                  for u in fillers:   # drain leftovers
                      u()
                  fillers.clear()

    nc.compile()
    return nc


_program = None


def _get_program():
    global _program
    if _program is None:
        _program = _build_program()
    return _program


def _make_tri():
    k = np.arange(128)[:, None]
    c = np.arange(128)[None, :]
    return (k <= c).astype(BF16)


def kernel(x, Wq, Wk, Wv, Wo):
    x = np.asarray(x, dtype=np.float32)
    Wq, Wk, Wv, Wo = (np.asarray(w, dtype=np.float32) for w in (Wq, Wk, Wv, Wo))
    nc = _get_program()

    xT = np.ascontiguousarray(x[0].T).astype(BF16)
    tri = _make_tri()
    ident = np.eye(128, dtype=BF16)
    in_maps = []
    for c in range(NCORES):
        hs = slice(128 * c, 128 * (c + 1))
        in_maps.append({
            "xT": xT,
            "wq": np.ascontiguousarray(Wq[:, hs]).astype(BF16),
            "wk": np.ascontiguousarray(Wk[:, hs]).astype(BF16),
            "wv": np.ascontiguousarray(Wv[:, hs]).astype(BF16),
            "wo": np.ascontiguousarray(Wo[hs, :]).astype(BF16),
            "tri": tri,
            "ident": ident,
        })

    res = run_bass_kernel_spmd(nc, in_maps, core_ids=list(range(NCORES)))
    out = np.zeros((S, D), np.float32)
    for c in range(NCORES):
        out += np.asarray(res.results[c]["y"], dtype=np.float32)
    return out.reshape(1, S, D)
